# revision 12
# baseline (speedup 1.0000x reference)
"""Trainium2 Bass kernel v4 for nn_Attention (topk_masking).

reference:
    h = tanh(x @ W1 + b1); e = h @ W2 + b2            # [B,T,1]
    thr = sort(e, axis=1)[:, T//2]
    mask: keep e < thr; softmax over kept; out = sum_t beta_t * x_t -> [B,D,1,1]

Strategy (per core, 4 samples):
  pass1: e~ = tanh(x16 @ W1_16 + b1) @ (W2 hi+lo fp16), single fp16 product;
         |e~ - e| <= ~4e-4 on this input distribution. DMA-bound (~73us).
  bisect: theta~ per sample, count(e~ < theta~) = 2048; four per-sample
         [32,128] DVE chains, interleaved so dependency stalls overlap and
         samples 0..2 hide under pass1's DMA window (13 iters from [-.5,.5]).
  boundary repair: elements with e~ in [theta~-DG, theta~+DG] are re-scored
         exactly: top-8 per 128-row by -(e~-theta~)^2 with definite-kept rows
         masked out, x rows gathered via SWDGE dma_gather (512-idx halves,
         transposed, fp16 hi+lo), MLP recomputed in near-fp32, exact K-th
         value selected by per-sample bisections. Reproduces the reference
         kept set exactly.
  pass2: S = sum_t u_t x_t on TensorE: u = exp(e~-theta~) masked (fp16),
         transposed per-sample on PE; one [128,1]x[128,512] matmul pair per
         128-row tile, x streamed t-major fp16 with deep prefetch; the PE
         stream is ordered [p1, MLP, pass2-mains, corrections] so the
         boundary machinery overlaps pass2. Corrections run in separate
         [1,512] psums and are added to the output on DVE. out = S / Z.

b2 is dropped (softmax shift-invariance).
"""
import os
import sys

sys.path.insert(0, "/opt/trn_rl_repo")

import numpy as np
import ml_dtypes  # noqa: F401

import concourse.bass as bass  # noqa: F401
from concourse import bacc
import concourse.tile as tile
import concourse.mybir as mybir
from concourse.bass_utils import run_bass_kernel_spmd

F32 = mybir.dt.float32
F16 = mybir.dt.float16
I16 = mybir.dt.int16
U16 = mybir.dt.uint16
U8 = mybir.dt.uint8
AF = mybir.ActivationFunctionType
ALU = mybir.AluOpType
AX = mybir.AxisListType

BSH, T, D, H = 4, 4096, 1024, 256
TT = 512
NEG_BIG = -99999999.0
DG = 1.2e-3          # boundary half-window on e~
VDG = 1.26e-3        # candidate-validity window (high side margin)
NIT_MAIN = int(os.environ.get("K_NIT_MAIN", "13"))
NIT_SEL = int(os.environ.get("K_NIT_SEL", "16"))
PHASE = int(os.environ.get("K_PHASE", "4"))
NCAND = 1024         # candidate slots total (4 samples x 32 rows x 8)
BCAST0 = [0] * 32


def build(repeat=1):
    nc = bacc.Bacc(trn_type="TRN2", target_bir_lowering=False)

    xT16 = nc.declare_dram_parameter("xT16", [BSH, 128, 8, T], F16, isOutput=False)
    xr16 = nc.declare_dram_parameter("xr16", [BSH * T, D], F16, isOutput=False)
    xr16l = nc.declare_dram_parameter("xr16l", [BSH * T, D], F16, isOutput=False)
    w1hp = nc.declare_dram_parameter("w1hp", [128, 8, H], F16, isOutput=False)
    w1lp = nc.declare_dram_parameter("w1lp", [128, 8, H], F16, isOutput=False)
    b1p = nc.declare_dram_parameter("b1p", [128, 2], F32, isOutput=False)
    w2hp = nc.declare_dram_parameter("w2hp", [128, 2], F16, isOutput=False)
    w2lp = nc.declare_dram_parameter("w2lp", [128, 2], F16, isOutput=False)
    w2fp = nc.declare_dram_parameter("w2fp", [128, 2], F32, isOutput=False)
    identp = nc.declare_dram_parameter("identp", [128, 32], F16, isOutput=False)
    out = nc.declare_dram_parameter("out", [BSH, D], F32, isOutput=True)

    with tile.TileContext(nc) as tc:
        with tc.tile_pool(name="w", bufs=1) as wpool, \
             tc.tile_pool(name="x", bufs=5) as xpool, \
             tc.tile_pool(name="h", bufs=4) as hpool, \
             tc.tile_pool(name="e", bufs=1) as epool, \
             tc.tile_pool(name="b", bufs=1) as bpool, \
             tc.tile_pool(name="g", bufs=1) as gpool, \
             tc.tile_pool(name="p2", bufs=10) as p2pool, \
             tc.tile_pool(name="o", bufs=1) as opool, \
             tc.tile_pool(name="ps", bufs=2, space="PSUM") as pspool, \
             tc.tile_pool(name="pse", bufs=2, space="PSUM") as psepool, \
             tc.tile_pool(name="ps2", bufs=1, space="PSUM") as ps2pool, \
             tc.tile_pool(name="dram", bufs=1, space="DRAM") as dpool:

            # ---- DRAM scratch ----
            e_dram = dpool.tile([BSH, T], F32, tag="e_dram")
            idx_dram = dpool.tile([1, NCAND], I16, tag="idx_dram")
            ec_dram = dpool.tile([1, NCAND], F32, tag="ec_dram")
            wc_dram = dpool.tile([1, NCAND], F16, tag="wc_dram")
            z_dram = dpool.tile([1, 4], F32, tag="z_dram")
            zb_dram = dpool.tile([1, 4], F32, tag="zb_dram")

            # ---- weights ----
            w1h = wpool.tile([128, 8, H], F16, tag="w1h")
            nc.sync.dma_start(w1h[:], w1hp.ap())
            w1l = wpool.tile([128, 8, H], F16, tag="w1l")
            nc.sync.dma_start(w1l[:], w1lp.ap())
            b1s = wpool.tile([128, 2], F32, tag="b1s")
            nc.sync.dma_start(b1s[:], b1p.ap())
            w2h = wpool.tile([128, 2], F16, tag="w2h")
            nc.sync.dma_start(w2h[:], w2hp.ap())
            w2l = wpool.tile([128, 2], F16, tag="w2l")
            nc.sync.dma_start(w2l[:], w2lp.ap())
            w2f = wpool.tile([128, 2], F32, tag="w2f")
            nc.sync.dma_start(w2f[:], w2fp.ap())
            ident = wpool.tile([128, 32], F16, tag="ident")
            nc.sync.dma_start(ident[:], identp.ap())

            rep_ctx = tc.For_i(0, repeat, 1) if repeat > 1 else None
            import contextlib
            with (rep_ctx if rep_ctx is not None else contextlib.nullcontext()):
                Eb4 = epool.tile([128, 128], F32, tag="Eb4")
                u4 = epool.tile([128, 128], F16, tag="u4")
                bT16 = epool.tile([128, 128], F16, tag="bT16")
                nbig = epool.tile([128, 128], F32, tag="nbig")
                nc.vector.memset(nbig[:], NEG_BIG)
                lo_t = bpool.tile([128, 1], F32, tag="lo")
                hi_t = bpool.tile([128, 1], F32, tag="hi")
                mid_t = bpool.tile([128, 1], F32, tag="mid")
                cmp_t = bpool.tile([128, 128], U8, tag="cmp")
                cscr_t = bpool.tile([128, 32], F32, tag="cscr")
                nc.vector.memset(cscr_t[:], 0.0)
                tot_t = bpool.tile([128, 1], F32, tag="tot")
                totb_t = bpool.tile([128, 1], F32, tag="totb")
                msk_t = bpool.tile([128, 1], U8, tag="msk")
                d_t = bpool.tile([128, 128], F32, tag="d")
                nd2_t = bpool.tile([128, 128], F32, tag="nd2")
                lowm_t = bpool.tile([128, 128], U8, tag="lowm")
                mx_t = bpool.tile([128, 8], F32, tag="mx")
                mi_t = bpool.tile([128, 8], U16, tag="mi")
                mi16_t = bpool.tile([128, 8], I16, tag="mi16")
                iot_t = bpool.tile([128, 8], I16, tag="iot")
                nc.gpsimd.iota(iot_t[:], pattern=[[0, 8]], base=0,
                               channel_multiplier=128)
                gidx_t = bpool.tile([128, 8], I16, tag="gidx")
                validf_t = bpool.tile([128, 8], F32, tag="validf")
                tts_t = bpool.tile([128, 1], F32, tag="tts")
                ntt_t = bpool.tile([128, 1], F32, tag="ntt")
                zscr_t = bpool.tile([128, 32], F32, tag="zscr")
                nc.vector.memset(zscr_t[:], 0.0)
                zt_t = bpool.tile([128, 1], F32, tag="zt")

                def emit_p1(b):
                    for ti in range(T // TT):
                        sl = slice(ti * TT, (ti + 1) * TT)
                        xh = xpool.tile([128, 8, TT], F16, tag="xh")
                        nc.sync.dma_start(xh[:], xT16.ap()[b, :, :, sl])
                        hs = []
                        for hh in range(2):
                            hsl = slice(hh * 128, (hh + 1) * 128)
                            ps = pspool.tile([128, TT], F32, tag="hps")
                            for dc in range(8):
                                nc.tensor.matmul(
                                    ps[:], w1h[:, dc, hsl], xh[:, dc, :],
                                    start=(dc == 0), stop=(dc == 7),
                                )
                            h16 = hpool.tile([128, TT], F16, tag="h16")
                            nc.scalar.activation(
                                h16[:], ps[:], AF.Tanh, bias=b1s[:, hh : hh + 1]
                            )
                            hs.append(h16)
                        eps = psepool.tile([1, TT], F32, tag="eps")
                        nc.tensor.matmul(eps[:], w2h[:, 0:1], hs[0][:], start=True, stop=False)
                        nc.tensor.matmul(eps[:], w2l[:, 0:1], hs[0][:], start=False, stop=False)
                        nc.tensor.matmul(eps[:], w2h[:, 1:2], hs[1][:], start=False, stop=False)
                        nc.tensor.matmul(eps[:], w2l[:, 1:2], hs[1][:], start=False, stop=True)
                        estage = hpool.tile([1, TT], F32, tag="estage")
                        nc.scalar.copy(estage[:], eps[:])
                        nc.sync.dma_start(e_dram[b : b + 1, sl], estage[:])

                # ---- per-sample machinery on [32b:32b+32, :] slices ----
                def emit_bisect_steps(b):
                    """Generator: one bisect chain for sample b, one op/yield."""
                    s3 = slice(32 * b, 32 * b + 32)
                    nc.sync.dma_start(
                        Eb4[s3, :], e_dram[b].rearrange("(lp f) -> lp f", lp=32)
                    )
                    Eb = Eb4[s3, :]
                    lo, hi, mid = lo_t[s3, :], hi_t[s3, :], mid_t[s3, :]
                    cmp_, cscr = cmp_t[s3, :], cscr_t[s3, :]
                    tot, totb, msk = tot_t[s3, :], totb_t[s3, :], msk_t[s3, :]
                    nc.vector.memset(lo, -0.5)
                    nc.vector.memset(hi, 0.5)
                    yield
                    for _ in range(NIT_MAIN):
                        nc.vector.tensor_scalar(mid, lo, hi, 0.5, ALU.add, ALU.mult)
                        yield
                        nc.vector.tensor_scalar(
                            cmp_, Eb, mid, 0.0, ALU.is_lt, ALU.add,
                            accum_out=cscr[:, 0:1],
                        )
                        yield
                        nc.vector.tensor_reduce(
                            tot, cscr, axis=AX.X, op=ALU.add, apply_transpose=True
                        )
                        yield
                        nc.vector.stream_shuffle(totb, tot, BCAST0)
                        yield
                        nc.vector.tensor_scalar(msk, totb, 2048.5, None, ALU.is_lt)
                        yield
                        nc.vector.copy_predicated(lo, msk, mid)
                        yield
                        nc.vector.tensor_scalar(msk, totb, 2048.5, None, ALU.is_ge)
                        yield
                        nc.vector.copy_predicated(hi, msk, mid)
                        yield

                def emit_cand_steps(b):
                    s3 = slice(32 * b, 32 * b + 32)
                    Eb = Eb4[s3, :]
                    lo = lo_t[s3, :]
                    tts = tts_t[s3, :]
                    nc.vector.tensor_scalar(tts, lo, -DG, None, ALU.add)
                    yield
                    d, nd2 = d_t[s3, :], nd2_t[s3, :]
                    nc.vector.tensor_scalar(d, Eb, lo, None, ALU.subtract)
                    yield
                    nc.vector.tensor_tensor(out=nd2, in0=d, in1=d, op=ALU.mult)
                    yield
                    nc.vector.tensor_scalar(nd2, nd2, -1.0, None, ALU.mult)
                    yield
                    lowm = lowm_t[s3, :]
                    nc.vector.tensor_scalar(
                        lowm, Eb, tts, 0.0, ALU.is_lt, ALU.add,
                        accum_out=cscr_t[s3, 0:1],
                    )
                    yield
                    nc.vector.copy_predicated(nd2, lowm, nbig[s3, :])
                    yield
                    nc.vector.tensor_reduce(
                        tot_t[s3, :], cscr_t[s3, :], axis=AX.X, op=ALU.add,
                        apply_transpose=True,
                    )
                    yield
                    mx, mi, mi16 = mx_t[s3, :], mi_t[s3, :], mi16_t[s3, :]
                    nc.vector.max(mx, nd2)
                    yield
                    nc.vector.max_index(mi, mx, nd2)
                    yield
                    nc.vector.tensor_scalar(validf_t[s3, :], mx, -VDG * VDG, None,
                                            ALU.is_ge)
                    yield
                    nc.vector.tensor_copy(mi16, mi)
                    yield
                    nc.vector.tensor_tensor(out=gidx_t[s3, :], in0=iot_t[s3, :],
                                            in1=mi16, op=ALU.add)
                    yield
                    nc.sync.dma_start(
                        idx_dram[0, 256 * b : 256 * b + 256].rearrange(
                            "(q s) -> q s", q=32),
                        gidx_t[s3, :],
                    )
                    yield

                def emit_softmax_steps(b):
                    s3 = slice(32 * b, 32 * b + 32)
                    Eb = Eb4[s3, :]
                    nc.vector.tensor_scalar(cmp_t[s3, :], Eb, tts_t[s3, :], None,
                                            ALU.is_ge)
                    yield
                    nc.vector.copy_predicated(Eb, cmp_t[s3, :], nbig[s3, :])
                    yield
                    nc.vector.tensor_scalar(ntt_t[s3, :], lo_t[s3, :], -1.0, None,
                                            ALU.mult)
                    yield
                    nc.scalar.activation(
                        u4[s3, :], Eb, AF.Exp,
                        bias=ntt_t[s3, :], scale=1.0, accum_out=zscr_t[s3, 0:1],
                    )
                    yield
                    nc.vector.tensor_reduce(
                        zt_t[s3, :], zscr_t[s3, :], axis=AX.X, op=ALU.add,
                        apply_transpose=True,
                    )
                    yield
                    nc.sync.dma_start(
                        z_dram[0:1, b : b + 1], zt_t[32 * b : 32 * b + 1, :]
                    )
                    pst = psepool.tile([128, 32], F16, tag="pst", bufs=2)
                    nc.tensor.transpose(pst[:], u4[s3, :], ident[s3, :],
                                        tile_position=(32 * b, 0))
                    yield
                    nc.scalar.copy(bT16[:, s3], pst[:])
                    yield

                def run_chains(chains):
                    """Round-robin the per-sample op generators (interleave)."""
                    live = list(chains)
                    while live:
                        nxt = []
                        for ch in live:
                            try:
                                next(ch)
                                nxt.append(ch)
                            except StopIteration:
                                pass
                        live = nxt

                def emit_gather_half(hf, idxs_sb):
                    ix = idxs_sb[:, 32 * hf : 32 * hf + 32]
                    a = gpool.tile([128, 8, 512], F16, tag="xgh", bufs=2,
                                   name=f"xgh{hf}")
                    nc.gpsimd.dma_gather(a[:], xr16.ap(), ix, 512, 512, D,
                                         transpose=True)
                    c = gpool.tile([128, 8, 512], F16, tag="xgl", bufs=2,
                                   name=f"xgl{hf}")
                    nc.gpsimd.dma_gather(c[:], xr16l.ap(), ix, 512, 512, D,
                                         transpose=True)
                    r = gpool.tile([128, 4, D], F16, tag="xgr", bufs=2,
                                   name=f"xgr{hf}")
                    nc.gpsimd.dma_gather(r[:], xr16.ap(), ix, 512, 512, D,
                                         transpose=False)
                    return a, c, r

                def emit_idxs(hf, idxs_sb):
                    # replicate [16, 32] half into the 8 16-partition groups
                    for k in range(8):
                        nc.sync.dma_start(
                            idxs_sb[16 * k : 16 * k + 16, 32 * hf : 32 * hf + 32],
                            idx_dram[0, 512 * hf : 512 * hf + 512].rearrange(
                                "(s p) -> p s", p=16),
                        )

                def emit_cand_mlp_half(hf, xgh, xgl, hcs, ecst):
                    cs = slice(hf * 512, (hf + 1) * 512)
                    for hh in range(2):
                        hsl = slice(hh * 128, (hh + 1) * 128)
                        ps = pspool.tile([128, 512], F32, tag="hps")
                        for dc in range(8):
                            nc.tensor.matmul(
                                ps[:], w1h[:, dc, hsl], xgh[:, dc, :],
                                start=(dc == 0), stop=False,
                            )
                            nc.tensor.matmul(
                                ps[:], w1l[:, dc, hsl], xgh[:, dc, :],
                                start=False, stop=False,
                            )
                            nc.tensor.matmul(
                                ps[:], w1h[:, dc, hsl], xgl[:, dc, :],
                                start=False, stop=(dc == 7),
                            )
                        nc.scalar.activation(
                            hcs[hh][:, cs], ps[:], AF.Tanh,
                            bias=b1s[:, hh : hh + 1]
                        )
                    ecps = psepool.tile([1, 512], F32, tag="eps")
                    nc.tensor.matmul(ecps[:], w2f[:, 0:1], hcs[0][:, cs],
                                     start=True, stop=False)
                    nc.tensor.matmul(ecps[:], w2f[:, 1:2], hcs[1][:, cs],
                                     start=False, stop=True)
                    nc.scalar.copy(ecst[:, cs], ecps[:])
                    nc.sync.dma_start(
                        ec_dram[0, cs].rearrange("(a i) -> a i", a=1),
                        ecst[:, cs],
                    )

                def emit_select_load():
                    ecn = bpool.tile([128, 8], F32, tag="ecn")
                    nc.sync.dma_start(ecn[:],
                                      ec_dram[0].rearrange("(p s) -> p s", p=128))
                    vm8 = bpool.tile([128, 8], U8, tag="vm8")
                    nc.vector.tensor_scalar(vm8[:], validf_t[:], 0.5, None,
                                            ALU.is_ge)
                    ecm = bpool.tile([128, 8], F32, tag="ecm")
                    nc.vector.memset(ecm[:], 1e9)
                    nc.vector.copy_predicated(ecm[:], vm8[:], ecn[:])
                    # per-sample kdef counts (at partitions 32b) -> broadcast
                    kb5 = bpool.tile([128, 1], F32, tag="kb5")
                    nc.vector.stream_shuffle(kb5[:], tot_t[:], BCAST0)
                    nc.vector.tensor_scalar(kb5[:], kb5[:], -1.0, 2048.5, ALU.mult,
                                            ALU.add)
                    lo2 = bpool.tile([128, 1], F32, tag="lo2")
                    nc.vector.tensor_scalar(lo2[:], lo_t[:], -(DG + 1e-6), None,
                                            ALU.add)
                    hi2 = bpool.tile([128, 1], F32, tag="hi2")
                    nc.vector.tensor_scalar(hi2[:], lo_t[:], DG + 1e-6, None,
                                            ALU.add)
                    scratch = (bpool.tile([128, 1], F32, tag="mid2", name="mid2"),
                               bpool.tile([128, 8], U8, tag="cmp2", name="cmp2"),
                               bpool.tile([128, 1], F32, tag="tot2", name="tot2"),
                               bpool.tile([128, 1], F32, tag="totb2", name="totb2"),
                               bpool.tile([128, 1], U8, tag="m2", name="m2s"))
                    return ecm, kb5, lo2, hi2, scratch

                def emit_select_steps(b, ecm, kb5, lo2, hi2, scratch):
                    s3 = slice(32 * b, 32 * b + 32)
                    ecmb, kb5b = ecm[s3, :], kb5[s3, :]
                    lob, hib = lo2[s3, :], hi2[s3, :]
                    mid2 = scratch[0][s3, :]
                    cmp2 = scratch[1][s3, :]
                    tot2 = scratch[2][s3, :]
                    totb2 = scratch[3][s3, :]
                    m2 = scratch[4][s3, :]
                    cscr = cscr_t[s3, :]
                    for _ in range(NIT_SEL):
                        nc.vector.tensor_scalar(mid2, lob, hib, 0.5, ALU.add,
                                                ALU.mult)
                        yield
                        nc.vector.tensor_scalar(
                            cmp2, ecmb, mid2, 0.0, ALU.is_lt, ALU.add,
                            accum_out=cscr[:, 0:1],
                        )
                        yield
                        nc.vector.tensor_reduce(
                            tot2, cscr, axis=AX.X, op=ALU.add, apply_transpose=True
                        )
                        yield
                        nc.vector.stream_shuffle(totb2, tot2, BCAST0)
                        yield
                        nc.vector.tensor_scalar(m2, totb2, kb5b, None, ALU.is_lt)
                        yield
                        nc.vector.copy_predicated(lob, m2, mid2)
                        yield
                        nc.vector.tensor_scalar(m2, totb2, kb5b, None, ALU.is_ge)
                        yield
                        nc.vector.copy_predicated(hib, m2, mid2)
                        yield

                def emit_select_finish(ecm, lo2):
                    dc8 = bpool.tile([128, 8], F32, tag="dc8")
                    nc.vector.tensor_scalar(dc8[:], ecm[:], ntt_t[:], 1.0, ALU.add,
                                            ALU.min)
                    uc8 = bpool.tile([128, 8], F32, tag="uc8")
                    nc.scalar.activation(uc8[:], dc8[:], AF.Exp)
                    selm = bpool.tile([128, 8], F32, tag="selm")
                    nc.vector.tensor_scalar(selm[:], ecm[:], lo2[:], None, ALU.is_lt)
                    wc8 = bpool.tile([128, 8], F32, tag="wc8")
                    nc.vector.tensor_tensor(out=wc8[:], in0=uc8[:], in1=selm[:],
                                            op=ALU.mult)
                    # Z_bnd per sample: row-reduce then 32-block transpose-reduce
                    nc.vector.tensor_scalar(
                        wc8[:], wc8[:], 1.0, 0.0, ALU.mult, ALU.add,
                        accum_out=cscr_t[:, 0:1],
                    )
                    zb1 = bpool.tile([128, 1], F32, tag="zb1")
                    nc.vector.tensor_reduce(zb1[:], cscr_t[:], axis=AX.X,
                                            op=ALU.add, apply_transpose=True)
                    for b in range(BSH):
                        nc.sync.dma_start(zb_dram[0:1, b : b + 1],
                                          zb1[32 * b : 32 * b + 1, :])
                    wc16 = bpool.tile([128, 8], F16, tag="wc16")
                    nc.vector.tensor_copy(wc16[:], wc8[:])
                    nc.sync.dma_start(wc_dram[0].rearrange("(q s) -> q s", q=128),
                                      wc16[:])
                    wcT = bpool.tile([128, 8], F16, tag="wcT")
                    nc.sync.dma_start(wcT[:],
                                      wc_dram[0].rearrange("(c p) -> p c", p=128))
                    return wcT

                state = {}
                ostages = []

                def emit_pass2_main(b):
                    S0 = ps2pool.tile([1, 512], F32, tag="S0", name=f"S0_{b}")
                    S1 = ps2pool.tile([1, 512], F32, tag="S1", name=f"S1_{b}")
                    for j in range(T // 128):
                        xt2 = p2pool.tile([128, D], F16, tag="xt2")
                        r0 = T * b + 128 * j
                        nc.sync.dma_start(xt2[:], xr16.ap()[r0 : r0 + 128, :])
                        col = 32 * b + j
                        nc.tensor.matmul(
                            S0[:], bT16[:, col : col + 1], xt2[:, 0:512],
                            start=(j == 0), stop=(j == 31),
                        )
                        nc.tensor.matmul(
                            S1[:], bT16[:, col : col + 1], xt2[:, 512:1024],
                            start=(j == 0), stop=(j == 31),
                        )
                    ost = opool.tile([1, D], F32, tag="ost", name=f"ost{b}")
                    nc.scalar.copy(ost[:, 0:512], S0[:])
                    nc.scalar.copy(ost[:, 512:1024], S1[:])
                    ostages.append(ost)

                def emit_pass2_corr(b):
                    wcT = state["wcT"]
                    xgr = state["xgr"][b // 2]
                    Sc0 = psepool.tile([1, 512], F32, tag="eps")
                    Sc1 = psepool.tile([1, 512], F32, tag="eps")
                    for k in range(2):
                        cc = 2 * b + k
                        lc = 2 * (b % 2) + k
                        nc.tensor.matmul(
                            Sc0[:], wcT[:, cc : cc + 1], xgr[:, lc, 0:512],
                            start=(k == 0), stop=(k == 1),
                        )
                        nc.tensor.matmul(
                            Sc1[:], wcT[:, cc : cc + 1], xgr[:, lc, 512:1024],
                            start=(k == 0), stop=(k == 1),
                        )
                    ost = ostages[b]
                    nc.vector.tensor_tensor(out=ost[:, 0:512], in0=ost[:, 0:512],
                                            in1=Sc0[:], op=ALU.add)
                    nc.vector.tensor_tensor(out=ost[:, 512:1024],
                                            in0=ost[:, 512:1024],
                                            in1=Sc1[:], op=ALU.add)

                def emit_endgame():
                    zm4 = opool.tile([1, 4], F32, tag="zm4")
                    nc.sync.dma_start(zm4[:], z_dram[0:1, :])
                    zb4 = opool.tile([1, 4], F32, tag="zb4")
                    nc.sync.dma_start(zb4[:], zb_dram[0:1, :])
                    zt4 = opool.tile([1, 4], F32, tag="zt4")
                    nc.vector.tensor_tensor(out=zt4[:], in0=zm4[:], in1=zb4[:],
                                            op=ALU.add)
                    rz4 = opool.tile([1, 4], F32, tag="rz4")
                    nc.vector.reciprocal(rz4[:], zt4[:])
                    for b in range(BSH):
                        fin = opool.tile([1, D], F32, tag="fin", name=f"fin{b}")
                        nc.scalar.activation(
                            fin[:], ostages[b][:], AF.Copy,
                            scale=rz4[0:1, b : b + 1],
                        )
                        nc.sync.dma_start(out.ap()[b], fin[:])

                # ---- schedule ----
                for b in range(BSH):
                    emit_p1(b)
                if PHASE >= 2:
                    run_chains([emit_bisect_steps(b) for b in range(BSH)])
                    run_chains([emit_cand_steps(b) for b in range(BSH)])
                    run_chains([emit_softmax_steps(b) for b in range(BSH)])
                if PHASE >= 3:
                    idxs_sb = gpool.tile([128, NCAND // 16], I16, tag="idxs")
                    ghs, gls, grs = [], [], []
                    for hf in range(2):
                        emit_idxs(hf, idxs_sb)
                        a, c, r = emit_gather_half(hf, idxs_sb)
                        ghs.append(a); gls.append(c); grs.append(r)
                    state["xgr"] = grs
                    hcs = [hpool.tile([128, NCAND], F32, tag="hc", name=f"hc{hh}")
                           for hh in range(2)]
                    ecst = hpool.tile([1, NCAND], F32, tag="ecst")
                    for hf in range(2):
                        emit_cand_mlp_half(hf, ghs[hf], gls[hf], hcs, ecst)
                    sel = emit_select_load()
                    run_chains([emit_select_steps(b, *sel) for b in range(BSH)])
                    state["wcT"] = emit_select_finish(sel[0], sel[2])
                if PHASE >= 4:
                    for b in range(BSH):
                        emit_pass2_main(b)
                    for b in range(BSH):
                        emit_pass2_corr(b)
                    emit_endgame()
                else:
                    zt_d = opool.tile([1, D], F32, tag="ztd")
                    nc.vector.memset(zt_d[:], float(PHASE))
                    for b in range(BSH):
                        nc.sync.dma_start(out.ap()[b], zt_d[:])
                ostages.clear()

    nc.finalize()
    return nc


_NC_CACHE = None


def _get_nc():
    global _NC_CACHE
    if _NC_CACHE is None:
        _NC_CACHE = build()
    return _NC_CACHE


def make_in_maps(x, W1, b1, W2, b2):
    del b2  # shift-invariant: no effect on the output
    x = np.asarray(x, dtype=np.float32)
    W1 = np.asarray(W1, dtype=np.float32)
    b1 = np.asarray(b1, dtype=np.float32).reshape(H)
    W2 = np.asarray(W2, dtype=np.float32).reshape(H)

    w1r = np.ascontiguousarray(W1.reshape(8, 128, H).transpose(1, 0, 2))
    w1hp = w1r.astype(np.float16)
    w1lp = (w1r - w1hp.astype(np.float32)).astype(np.float16)
    b1p = np.ascontiguousarray(b1.reshape(2, 128).T)
    w2r = np.ascontiguousarray(W2.reshape(2, 128).T)
    w2hp = w2r.astype(np.float16)
    w2lp = (w2r - w2hp.astype(np.float32)).astype(np.float16)
    identp = np.tile(np.eye(32, dtype=np.float16), (4, 1))

    in_maps = []
    for c in range(8):
        xs = x[4 * c : 4 * c + 4]  # [4, T, D]
        xh = xs.astype(np.float16)
        xlo = (xs - xh.astype(np.float32)).astype(np.float16)
        xt = np.ascontiguousarray(
            xh.transpose(0, 2, 1).reshape(BSH, 8, 128, T).transpose(0, 2, 1, 3)
        )  # [4, 128, 8, T]; xt[b,p,dc,t] = xh[b,t,dc*128+p]
        in_maps.append(
            {
                "xT16": xt,
                "xr16": np.ascontiguousarray(xh.reshape(BSH * T, D)),
                "xr16l": np.ascontiguousarray(xlo.reshape(BSH * T, D)),
                "w1hp": w1hp,
                "w1lp": w1lp,
                "b1p": b1p,
                "w2hp": w2hp,
                "w2lp": w2lp,
                "w2fp": w2r,
                "identp": identp,
            }
        )
    return in_maps


def kernel(x, W1, b1, W2, b2):
    nc = _get_nc()
    in_maps = make_in_maps(x, W1, b1, W2, b2)
    res = run_bass_kernel_spmd(nc, in_maps, core_ids=list(range(8)))
    outs = [res.results[c]["out"] for c in range(8)]
    full = np.concatenate(outs, axis=0).astype(np.float32)  # [32, 1024]
    return full[:, :, None, None]


# revision 14
# speedup vs baseline: 1.0288x; 1.0288x over previous
"""Trainium2 Bass kernel v4 for nn_Attention (topk_masking).

reference:
    h = tanh(x @ W1 + b1); e = h @ W2 + b2            # [B,T,1]
    thr = sort(e, axis=1)[:, T//2]
    mask: keep e < thr; softmax over kept; out = sum_t beta_t * x_t -> [B,D,1,1]

Strategy (per core, 4 samples):
  pass1: e~ = tanh(x16 @ W1_16 + b1) @ (W2 hi+lo fp16), single fp16 product;
         |e~ - e| <= ~4e-4 on this input distribution. DMA-bound (~73us).
  bisect: theta~ per sample, count(e~ < theta~) = 2048; four per-sample
         [32,128] DVE chains, interleaved so dependency stalls overlap and
         samples 0..2 hide under pass1's DMA window (13 iters from [-.5,.5]).
  boundary repair: elements with e~ in [theta~-DG, theta~+DG] are re-scored
         exactly: top-8 per 128-row by -(e~-theta~)^2 with definite-kept rows
         masked out, x rows gathered via SWDGE dma_gather (512-idx halves,
         transposed, fp16 hi+lo), MLP recomputed in near-fp32, exact K-th
         value selected by per-sample bisections. Reproduces the reference
         kept set exactly.
  pass2: S = sum_t u_t x_t on TensorE: u = exp(e~-theta~) masked (fp16),
         transposed per-sample on PE; one [128,1]x[128,512] matmul pair per
         128-row tile, x streamed t-major fp16 with deep prefetch; the PE
         stream is ordered [p1, MLP, pass2-mains, corrections] so the
         boundary machinery overlaps pass2. Corrections run in separate
         [1,512] psums and are added to the output on DVE. out = S / Z.

b2 is dropped (softmax shift-invariance).
"""
import os
import sys

sys.path.insert(0, "/opt/trn_rl_repo")

import numpy as np
import ml_dtypes  # noqa: F401

import concourse.bass as bass  # noqa: F401
from concourse import bacc
import concourse.tile as tile
import concourse.mybir as mybir
from concourse.bass_utils import run_bass_kernel_spmd

F32 = mybir.dt.float32
F16 = mybir.dt.float16
I16 = mybir.dt.int16
U16 = mybir.dt.uint16
U8 = mybir.dt.uint8
AF = mybir.ActivationFunctionType
ALU = mybir.AluOpType
AX = mybir.AxisListType

BSH, T, D, H = 4, 4096, 1024, 256
TT = 512
NEG_BIG = -99999999.0
DG = 1.2e-3          # boundary half-window on e~
VDG = 1.26e-3        # candidate-validity window (high side margin)
NIT_MAIN = int(os.environ.get("K_NIT_MAIN", "13"))
NIT_SEL = int(os.environ.get("K_NIT_SEL", "16"))
PHASE = int(os.environ.get("K_PHASE", "4"))
NCAND = 1024         # candidate slots total (4 samples x 32 rows x 8)
BCAST0 = [0] * 32


def build(repeat=1):
    nc = bacc.Bacc(trn_type="TRN2", target_bir_lowering=False)

    xT16 = nc.declare_dram_parameter("xT16", [BSH, 128, 8, T], F16, isOutput=False)
    xr16 = nc.declare_dram_parameter("xr16", [BSH * T, D], F16, isOutput=False)
    xr16l = nc.declare_dram_parameter("xr16l", [BSH * T, D], F16, isOutput=False)
    w1hp = nc.declare_dram_parameter("w1hp", [128, 8, H], F16, isOutput=False)
    w1lp = nc.declare_dram_parameter("w1lp", [128, 8, H], F16, isOutput=False)
    b1p = nc.declare_dram_parameter("b1p", [128, 2], F32, isOutput=False)
    w2hp = nc.declare_dram_parameter("w2hp", [128, 2], F16, isOutput=False)
    w2lp = nc.declare_dram_parameter("w2lp", [128, 2], F16, isOutput=False)
    w2fp = nc.declare_dram_parameter("w2fp", [128, 2], F32, isOutput=False)
    identp = nc.declare_dram_parameter("identp", [128, 32], F16, isOutput=False)
    out = nc.declare_dram_parameter("out", [BSH, D], F32, isOutput=True)

    with tile.TileContext(nc) as tc:
        with tc.tile_pool(name="w", bufs=1) as wpool, \
             tc.tile_pool(name="x", bufs=5) as xpool, \
             tc.tile_pool(name="h", bufs=4) as hpool, \
             tc.tile_pool(name="e", bufs=1) as epool, \
             tc.tile_pool(name="b", bufs=1) as bpool, \
             tc.tile_pool(name="g", bufs=1) as gpool, \
             tc.tile_pool(name="p2", bufs=10) as p2pool, \
             tc.tile_pool(name="o", bufs=1) as opool, \
             tc.tile_pool(name="ps", bufs=2, space="PSUM") as pspool, \
             tc.tile_pool(name="pse", bufs=2, space="PSUM") as psepool, \
             tc.tile_pool(name="ps2", bufs=1, space="PSUM") as ps2pool, \
             tc.tile_pool(name="dram", bufs=1, space="DRAM") as dpool:

            # ---- DRAM scratch ----
            e_dram = dpool.tile([BSH, T], F32, tag="e_dram")
            idx_dram = dpool.tile([1, NCAND], I16, tag="idx_dram")
            ec_dram = dpool.tile([1, NCAND], F32, tag="ec_dram")
            wc_dram = dpool.tile([1, NCAND], F16, tag="wc_dram")
            z_dram = dpool.tile([1, 4], F32, tag="z_dram")
            zb_dram = dpool.tile([1, 4], F32, tag="zb_dram")

            # ---- weights ----
            w1h = wpool.tile([128, 8, H], F16, tag="w1h")
            nc.sync.dma_start(w1h[:], w1hp.ap())
            w1l = wpool.tile([128, 8, H], F16, tag="w1l")
            nc.sync.dma_start(w1l[:], w1lp.ap())
            b1s = wpool.tile([128, 2], F32, tag="b1s")
            nc.sync.dma_start(b1s[:], b1p.ap())
            w2h = wpool.tile([128, 2], F16, tag="w2h")
            nc.sync.dma_start(w2h[:], w2hp.ap())
            w2l = wpool.tile([128, 2], F16, tag="w2l")
            nc.sync.dma_start(w2l[:], w2lp.ap())
            w2f = wpool.tile([128, 2], F32, tag="w2f")
            nc.sync.dma_start(w2f[:], w2fp.ap())
            ident = wpool.tile([128, 32], F16, tag="ident")
            nc.sync.dma_start(ident[:], identp.ap())

            rep_ctx = tc.For_i(0, repeat, 1) if repeat > 1 else None
            import contextlib
            with (rep_ctx if rep_ctx is not None else contextlib.nullcontext()):
                Eb4 = epool.tile([128, 128], F32, tag="Eb4")
                u4 = epool.tile([128, 128], F16, tag="u4")
                bT16 = epool.tile([128, 128], F16, tag="bT16")
                nbig = epool.tile([128, 128], F32, tag="nbig")
                nc.vector.memset(nbig[:], NEG_BIG)
                lo_t = bpool.tile([128, 1], F32, tag="lo")
                hi_t = bpool.tile([128, 1], F32, tag="hi")
                mid_t = bpool.tile([128, 1], F32, tag="mid")
                cmp_t = bpool.tile([128, 128], U8, tag="cmp")
                cscr_t = bpool.tile([128, 32], F32, tag="cscr")
                nc.vector.memset(cscr_t[:], 0.0)
                tot_t = bpool.tile([128, 1], F32, tag="tot")
                totb_t = bpool.tile([128, 1], F32, tag="totb")
                msk_t = bpool.tile([128, 1], U8, tag="msk")
                d_t = bpool.tile([128, 128], F32, tag="d")
                nd2_t = bpool.tile([128, 128], F32, tag="nd2")
                lowm_t = bpool.tile([128, 128], U8, tag="lowm")
                mx_t = bpool.tile([128, 8], F32, tag="mx")
                mi_t = bpool.tile([128, 8], U16, tag="mi")
                mi16_t = bpool.tile([128, 8], I16, tag="mi16")
                iot_t = bpool.tile([128, 8], I16, tag="iot")
                nc.gpsimd.iota(iot_t[:], pattern=[[0, 8]], base=0,
                               channel_multiplier=128)
                gidx_t = bpool.tile([128, 8], I16, tag="gidx")
                validf_t = bpool.tile([128, 8], F32, tag="validf")
                tts_t = bpool.tile([128, 1], F32, tag="tts")
                ntt_t = bpool.tile([128, 1], F32, tag="ntt")
                zscr_t = bpool.tile([128, 32], F32, tag="zscr")
                nc.vector.memset(zscr_t[:], 0.0)
                zt_t = bpool.tile([128, 1], F32, tag="zt")

                def emit_p1(b):
                    for ti in range(T // TT):
                        sl = slice(ti * TT, (ti + 1) * TT)
                        xh = xpool.tile([128, 8, TT], F16, tag="xh")
                        nc.sync.dma_start(xh[:], xT16.ap()[b, :, :, sl])
                        hs = []
                        for hh in range(2):
                            hsl = slice(hh * 128, (hh + 1) * 128)
                            ps = pspool.tile([128, TT], F32, tag="hps")
                            for dc in range(8):
                                nc.tensor.matmul(
                                    ps[:], w1h[:, dc, hsl], xh[:, dc, :],
                                    start=(dc == 0), stop=(dc == 7),
                                )
                            h16 = hpool.tile([128, TT], F16, tag="h16")
                            nc.scalar.activation(
                                h16[:], ps[:], AF.Tanh, bias=b1s[:, hh : hh + 1]
                            )
                            hs.append(h16)
                        eps = psepool.tile([1, TT], F32, tag="eps")
                        nc.tensor.matmul(eps[:], w2h[:, 0:1], hs[0][:], start=True, stop=False)
                        nc.tensor.matmul(eps[:], w2l[:, 0:1], hs[0][:], start=False, stop=False)
                        nc.tensor.matmul(eps[:], w2h[:, 1:2], hs[1][:], start=False, stop=False)
                        nc.tensor.matmul(eps[:], w2l[:, 1:2], hs[1][:], start=False, stop=True)
                        estage = hpool.tile([1, TT], F32, tag="estage")
                        nc.scalar.copy(estage[:], eps[:])
                        nc.sync.dma_start(e_dram[b : b + 1, sl], estage[:])

                # ---- merged machinery: multiway search over [128,128] ----
                ramp_i = bpool.tile([128, 15], I16, tag="ramp_i")
                nc.gpsimd.iota(ramp_i[:], pattern=[[1, 15]], base=1,
                               channel_multiplier=0)
                ramp_f = bpool.tile([128, 15], F32, tag="ramp_f")
                nc.vector.tensor_copy(ramp_f[:], ramp_i[:])
                vv_t = bpool.tile([128, 15], F32, tag="vv")
                idxb_t = bpool.tile([128, 1], F32, tag="idxb")
                step_t = bpool.tile([128, 1], F32, tag="step")

                cmp8_t = bpool.tile([128, 8], U8, tag="cmp8")

                def mw_stage(vals, cmpo, lo, delta, target, target_ap):
                    """One 16-way refinement: lo <- lo + idx*delta."""
                    nc.vector.tensor_scalar(vv_t[:], ramp_f[:], delta, None,
                                            ALU.mult)
                    nc.vector.tensor_scalar(vv_t[:], vv_t[:], lo, None, ALU.add)
                    for i in range(15):
                        nc.vector.tensor_scalar(
                            cmpo, vals, vv_t[:, i : i + 1], 0.0,
                            ALU.is_lt, ALU.add, accum_out=cscr_t[:, i : i + 1],
                        )
                    nc.vector.tensor_reduce(
                        tot_t[:], cscr_t[:], axis=AX.X, op=ALU.add,
                        apply_transpose=True,
                    )
                    tgt = target if target_ap is None else target_ap
                    nc.vector.tensor_scalar(cscr_t[:, 0:1], tot_t[:], tgt,
                                            None, ALU.is_lt)
                    # zero stale accum cols so later stages stay clean
                    for i in range(1, 15):
                        nc.vector.memset(cscr_t[:, i : i + 1], 0.0)
                    nc.vector.tensor_reduce(
                        totb_t[:], cscr_t[:], axis=AX.X, op=ALU.add,
                        apply_transpose=True,
                    )
                    nc.vector.stream_shuffle(idxb_t[:], totb_t[:], BCAST0)
                    # blocksum counts 17 always-true rows (j=15..31)
                    nc.vector.tensor_scalar(step_t[:], idxb_t[:], delta,
                                            -17.0 * delta, ALU.mult, ALU.add)
                    nc.vector.tensor_tensor(out=lo, in0=lo, in1=step_t[:],
                                            op=ALU.add)

                def emit_bisect():
                    for b in range(BSH):
                        nc.sync.dma_start(
                            Eb4[32 * b : 32 * b + 32, :],
                            e_dram[b].rearrange("(lp f) -> lp f", lp=32),
                        )
                    nc.vector.memset(lo_t[:], -0.5)
                    w = 1.0
                    for _ in range(3):
                        w /= 16.0
                        mw_stage(Eb4[:], cmp_t[:], lo_t[:], w, 2048.5, None)

                def emit_cand():
                    nc.vector.tensor_scalar(tts_t[:], lo_t[:], -DG, None, ALU.add)
                    nc.vector.tensor_scalar(d_t[:], Eb4[:], lo_t[:], None,
                                            ALU.subtract)
                    nc.vector.tensor_tensor(out=nd2_t[:], in0=d_t[:], in1=d_t[:],
                                            op=ALU.mult)
                    nc.vector.tensor_scalar(nd2_t[:], nd2_t[:], -1.0, None,
                                            ALU.mult)
                    nc.vector.tensor_scalar(
                        lowm_t[:], Eb4[:], tts_t[:], 0.0, ALU.is_lt, ALU.add,
                        accum_out=cscr_t[:, 0:1],
                    )
                    nc.vector.copy_predicated(nd2_t[:], lowm_t[:], nbig[:])
                    nc.vector.tensor_reduce(
                        tot_t[:], cscr_t[:], axis=AX.X, op=ALU.add,
                        apply_transpose=True,
                    )
                    nc.vector.memset(cscr_t[:, 0:1], 0.0)
                    nc.vector.max(mx_t[:], nd2_t[:])
                    nc.vector.max_index(mi_t[:], mx_t[:], nd2_t[:])
                    nc.vector.tensor_scalar(validf_t[:], mx_t[:], -VDG * VDG,
                                            None, ALU.is_ge)
                    nc.vector.tensor_copy(mi16_t[:], mi_t[:])
                    nc.vector.tensor_tensor(out=gidx_t[:], in0=iot_t[:],
                                            in1=mi16_t[:], op=ALU.add)
                    for hf in range(2):
                        nc.sync.dma_start(
                            idx_dram[0, 512 * hf : 512 * hf + 512].rearrange(
                                "(q s) -> q s", q=64),
                            gidx_t[64 * hf : 64 * hf + 64, :],
                        )

                def emit_softmax():
                    nc.vector.tensor_scalar(cmp_t[:], Eb4[:], tts_t[:], None,
                                            ALU.is_ge)
                    nc.vector.copy_predicated(Eb4[:], cmp_t[:], nbig[:])
                    nc.vector.tensor_scalar(ntt_t[:], lo_t[:], -1.0, None,
                                            ALU.mult)
                    nc.scalar.activation(
                        u4[:], Eb4[:], AF.Exp,
                        bias=ntt_t[:], scale=1.0, accum_out=zscr_t[:, 0:1],
                    )
                    nc.vector.tensor_reduce(
                        zt_t[:], zscr_t[:], axis=AX.X, op=ALU.add,
                        apply_transpose=True,
                    )
                    for b in range(BSH):
                        nc.sync.dma_start(
                            z_dram[0:1, b : b + 1], zt_t[32 * b : 32 * b + 1, :]
                        )
                    for b in range(BSH):
                        s3 = slice(32 * b, 32 * b + 32)
                        pst = psepool.tile([128, 32], F16, tag="pst", bufs=2)
                        nc.tensor.transpose(pst[:], u4[s3, :], ident[s3, :],
                                            tile_position=(32 * b, 0))
                        nc.scalar.copy(bT16[:, s3], pst[:])

                def emit_gather_half(hf, idxs_sb):
                    ix = idxs_sb[:, 32 * hf : 32 * hf + 32]
                    a = gpool.tile([128, 8, 512], F16, tag="xgh", bufs=2,
                                   name=f"xgh{hf}")
                    nc.gpsimd.dma_gather(a[:], xr16.ap(), ix, 512, 512, D,
                                         transpose=True)
                    c = gpool.tile([128, 8, 512], F16, tag="xgl", bufs=2,
                                   name=f"xgl{hf}")
                    nc.gpsimd.dma_gather(c[:], xr16l.ap(), ix, 512, 512, D,
                                         transpose=True)
                    r = gpool.tile([128, 4, D], F16, tag="xgr", bufs=2,
                                   name=f"xgr{hf}")
                    nc.gpsimd.dma_gather(r[:], xr16.ap(), ix, 512, 512, D,
                                         transpose=False)
                    return a, c, r

                def emit_idxs(hf, idxs_sb):
                    # replicate [16, 32] half into the 8 16-partition groups
                    for k in range(8):
                        nc.sync.dma_start(
                            idxs_sb[16 * k : 16 * k + 16, 32 * hf : 32 * hf + 32],
                            idx_dram[0, 512 * hf : 512 * hf + 512].rearrange(
                                "(s p) -> p s", p=16),
                        )

                def emit_cand_mlp_half(hf, xgh, xgl, hcs, ecst):
                    cs = slice(hf * 512, (hf + 1) * 512)
                    for hh in range(2):
                        hsl = slice(hh * 128, (hh + 1) * 128)
                        ps = pspool.tile([128, 512], F32, tag="hps")
                        for dc in range(8):
                            nc.tensor.matmul(
                                ps[:], w1h[:, dc, hsl], xgh[:, dc, :],
                                start=(dc == 0), stop=False,
                            )
                            nc.tensor.matmul(
                                ps[:], w1l[:, dc, hsl], xgh[:, dc, :],
                                start=False, stop=False,
                            )
                            nc.tensor.matmul(
                                ps[:], w1h[:, dc, hsl], xgl[:, dc, :],
                                start=False, stop=(dc == 7),
                            )
                        nc.scalar.activation(
                            hcs[hh][:, cs], ps[:], AF.Tanh,
                            bias=b1s[:, hh : hh + 1]
                        )
                    ecps = psepool.tile([1, 512], F32, tag="eps")
                    nc.tensor.matmul(ecps[:], w2f[:, 0:1], hcs[0][:, cs],
                                     start=True, stop=False)
                    nc.tensor.matmul(ecps[:], w2f[:, 1:2], hcs[1][:, cs],
                                     start=False, stop=True)
                    nc.scalar.copy(ecst[:, cs], ecps[:])
                    nc.sync.dma_start(
                        ec_dram[0, cs].rearrange("(a i) -> a i", a=1),
                        ecst[:, cs],
                    )

                def emit_select():
                    ecn = bpool.tile([128, 8], F32, tag="ecn")
                    nc.sync.dma_start(ecn[:],
                                      ec_dram[0].rearrange("(p s) -> p s", p=128))
                    vm8 = bpool.tile([128, 8], U8, tag="vm8")
                    nc.vector.tensor_scalar(vm8[:], validf_t[:], 0.5, None,
                                            ALU.is_ge)
                    ecm = bpool.tile([128, 8], F32, tag="ecm")
                    nc.vector.memset(ecm[:], 1e9)
                    nc.vector.copy_predicated(ecm[:], vm8[:], ecn[:])
                    # per-sample boundary-kept target (at partitions 32b) -> bcast
                    kb5 = bpool.tile([128, 1], F32, tag="kb5")
                    nc.vector.stream_shuffle(kb5[:], tot_t[:], BCAST0)
                    nc.vector.tensor_scalar(kb5[:], kb5[:], -1.0, 2048.5, ALU.mult,
                                            ALU.add)
                    lo2 = bpool.tile([128, 1], F32, tag="lo2")
                    nc.vector.tensor_scalar(lo2[:], lo_t[:], -(DG + 1e-6), None,
                                            ALU.add)
                    w = 2.0 * (DG + 1e-6)
                    for _ in range(4):
                        w /= 16.0
                        mw_stage(ecm[:], cmp8_t[:], lo2[:], w, None, kb5[:])
                    return ecm, lo2

                def emit_select_finish(ecm, lo2):
                    dc8 = bpool.tile([128, 8], F32, tag="dc8")
                    nc.vector.tensor_scalar(dc8[:], ecm[:], ntt_t[:], 1.0, ALU.add,
                                            ALU.min)
                    uc8 = bpool.tile([128, 8], F32, tag="uc8")
                    nc.scalar.activation(uc8[:], dc8[:], AF.Exp)
                    selm = bpool.tile([128, 8], F32, tag="selm")
                    nc.vector.tensor_scalar(selm[:], ecm[:], lo2[:], None, ALU.is_lt)
                    wc8 = bpool.tile([128, 8], F32, tag="wc8")
                    nc.vector.tensor_tensor(out=wc8[:], in0=uc8[:], in1=selm[:],
                                            op=ALU.mult)
                    # Z_bnd per sample: row-reduce then 32-block transpose-reduce
                    nc.vector.tensor_scalar(
                        wc8[:], wc8[:], 1.0, 0.0, ALU.mult, ALU.add,
                        accum_out=cscr_t[:, 0:1],
                    )
                    zb1 = bpool.tile([128, 1], F32, tag="zb1")
                    nc.vector.tensor_reduce(zb1[:], cscr_t[:], axis=AX.X,
                                            op=ALU.add, apply_transpose=True)
                    for b in range(BSH):
                        nc.sync.dma_start(zb_dram[0:1, b : b + 1],
                                          zb1[32 * b : 32 * b + 1, :])
                    wc16 = bpool.tile([128, 8], F16, tag="wc16")
                    nc.vector.tensor_copy(wc16[:], wc8[:])
                    nc.sync.dma_start(wc_dram[0].rearrange("(q s) -> q s", q=128),
                                      wc16[:])
                    wcT = bpool.tile([128, 8], F16, tag="wcT")
                    nc.sync.dma_start(wcT[:],
                                      wc_dram[0].rearrange("(c p) -> p c", p=128))
                    return wcT

                state = {}
                ostages = []

                def emit_pass2_main(b):
                    S0 = ps2pool.tile([1, 512], F32, tag="S0", name=f"S0_{b}")
                    S1 = ps2pool.tile([1, 512], F32, tag="S1", name=f"S1_{b}")
                    for j in range(T // 128):
                        xt2 = p2pool.tile([128, D], F16, tag="xt2")
                        r0 = T * b + 128 * j
                        nc.sync.dma_start(xt2[:], xr16.ap()[r0 : r0 + 128, :])
                        col = 32 * b + j
                        nc.tensor.matmul(
                            S0[:], bT16[:, col : col + 1], xt2[:, 0:512],
                            start=(j == 0), stop=(j == 31),
                        )
                        nc.tensor.matmul(
                            S1[:], bT16[:, col : col + 1], xt2[:, 512:1024],
                            start=(j == 0), stop=(j == 31),
                        )
                    ost = opool.tile([1, D], F32, tag="ost", name=f"ost{b}")
                    nc.scalar.copy(ost[:, 0:512], S0[:])
                    nc.scalar.copy(ost[:, 512:1024], S1[:])
                    ostages.append(ost)

                def emit_pass2_corr(b):
                    wcT = state["wcT"]
                    xgr = state["xgr"][b // 2]
                    Sc0 = psepool.tile([1, 512], F32, tag="eps")
                    Sc1 = psepool.tile([1, 512], F32, tag="eps")
                    for k in range(2):
                        cc = 2 * b + k
                        lc = 2 * (b % 2) + k
                        nc.tensor.matmul(
                            Sc0[:], wcT[:, cc : cc + 1], xgr[:, lc, 0:512],
                            start=(k == 0), stop=(k == 1),
                        )
                        nc.tensor.matmul(
                            Sc1[:], wcT[:, cc : cc + 1], xgr[:, lc, 512:1024],
                            start=(k == 0), stop=(k == 1),
                        )
                    ost = ostages[b]
                    nc.vector.tensor_tensor(out=ost[:, 0:512], in0=ost[:, 0:512],
                                            in1=Sc0[:], op=ALU.add)
                    nc.vector.tensor_tensor(out=ost[:, 512:1024],
                                            in0=ost[:, 512:1024],
                                            in1=Sc1[:], op=ALU.add)

                def emit_endgame():
                    zm4 = opool.tile([1, 4], F32, tag="zm4")
                    nc.sync.dma_start(zm4[:], z_dram[0:1, :])
                    zb4 = opool.tile([1, 4], F32, tag="zb4")
                    nc.sync.dma_start(zb4[:], zb_dram[0:1, :])
                    zt4 = opool.tile([1, 4], F32, tag="zt4")
                    nc.vector.tensor_tensor(out=zt4[:], in0=zm4[:], in1=zb4[:],
                                            op=ALU.add)
                    rz4 = opool.tile([1, 4], F32, tag="rz4")
                    nc.vector.reciprocal(rz4[:], zt4[:])
                    for b in range(BSH):
                        fin = opool.tile([1, D], F32, tag="fin", name=f"fin{b}")
                        nc.scalar.activation(
                            fin[:], ostages[b][:], AF.Copy,
                            scale=rz4[0:1, b : b + 1],
                        )
                        nc.sync.dma_start(out.ap()[b], fin[:])

                # ---- schedule ----
                for b in range(BSH):
                    emit_p1(b)
                if PHASE >= 2:
                    emit_bisect()
                    emit_cand()
                    emit_softmax()
                if PHASE >= 3:
                    idxs_sb = gpool.tile([128, NCAND // 16], I16, tag="idxs")
                    ghs, gls, grs = [], [], []
                    for hf in range(2):
                        emit_idxs(hf, idxs_sb)
                        a, c, r = emit_gather_half(hf, idxs_sb)
                        ghs.append(a); gls.append(c); grs.append(r)
                    state["xgr"] = grs
                    hcs = [hpool.tile([128, NCAND], F32, tag="hc", name=f"hc{hh}")
                           for hh in range(2)]
                    ecst = hpool.tile([1, NCAND], F32, tag="ecst")
                    for hf in range(2):
                        emit_cand_mlp_half(hf, ghs[hf], gls[hf], hcs, ecst)
                    sel = emit_select()
                    state["wcT"] = emit_select_finish(sel[0], sel[1])
                if PHASE >= 4:
                    for b in range(BSH):
                        emit_pass2_main(b)
                    for b in range(BSH):
                        emit_pass2_corr(b)
                    emit_endgame()
                else:
                    zt_d = opool.tile([1, D], F32, tag="ztd")
                    nc.vector.memset(zt_d[:], float(PHASE))
                    for b in range(BSH):
                        nc.sync.dma_start(out.ap()[b], zt_d[:])
                ostages.clear()

    nc.finalize()
    return nc


_NC_CACHE = None


def _get_nc():
    global _NC_CACHE
    if _NC_CACHE is None:
        _NC_CACHE = build()
    return _NC_CACHE


def make_in_maps(x, W1, b1, W2, b2):
    del b2  # shift-invariant: no effect on the output
    x = np.asarray(x, dtype=np.float32)
    W1 = np.asarray(W1, dtype=np.float32)
    b1 = np.asarray(b1, dtype=np.float32).reshape(H)
    W2 = np.asarray(W2, dtype=np.float32).reshape(H)

    w1r = np.ascontiguousarray(W1.reshape(8, 128, H).transpose(1, 0, 2))
    w1hp = w1r.astype(np.float16)
    w1lp = (w1r - w1hp.astype(np.float32)).astype(np.float16)
    b1p = np.ascontiguousarray(b1.reshape(2, 128).T)
    w2r = np.ascontiguousarray(W2.reshape(2, 128).T)
    w2hp = w2r.astype(np.float16)
    w2lp = (w2r - w2hp.astype(np.float32)).astype(np.float16)
    identp = np.tile(np.eye(32, dtype=np.float16), (4, 1))

    in_maps = []
    for c in range(8):
        xs = x[4 * c : 4 * c + 4]  # [4, T, D]
        xh = xs.astype(np.float16)
        xlo = (xs - xh.astype(np.float32)).astype(np.float16)
        xt = np.ascontiguousarray(
            xh.transpose(0, 2, 1).reshape(BSH, 8, 128, T).transpose(0, 2, 1, 3)
        )  # [4, 128, 8, T]; xt[b,p,dc,t] = xh[b,t,dc*128+p]
        in_maps.append(
            {
                "xT16": xt,
                "xr16": np.ascontiguousarray(xh.reshape(BSH * T, D)),
                "xr16l": np.ascontiguousarray(xlo.reshape(BSH * T, D)),
                "w1hp": w1hp,
                "w1lp": w1lp,
                "b1p": b1p,
                "w2hp": w2hp,
                "w2lp": w2lp,
                "w2fp": w2r,
                "identp": identp,
            }
        )
    return in_maps


def kernel(x, W1, b1, W2, b2):
    nc = _get_nc()
    in_maps = make_in_maps(x, W1, b1, W2, b2)
    res = run_bass_kernel_spmd(nc, in_maps, core_ids=list(range(8)))
    outs = [res.results[c]["out"] for c in range(8)]
    full = np.concatenate(outs, axis=0).astype(np.float32)  # [32, 1024]
    return full[:, :, None, None]
